# revision 13
# baseline (speedup 1.0000x reference)
"""DimeNet-style angle feature extractor on 8 Trainium2 NeuronCores.

Strategy (per the sharding hint's "shard edges and pre-bucket triplets by
owning edge" option):
  * Core c owns edges [c*62500, (c+1)*62500).
  * Host buckets triplets by the owning core of id_reduce_ji, relabels each
    core's local edges sorted by triplet-count (descending), and layers the
    triplets into rank-slots: slot s holds the rank-s triplet of edge e at
    stream position e.  Because edges are sorted by count, slot s is an
    exact prefix [0, c_s) -- the segment-sum becomes dense prefix adds.
  * id_expand_kj is pre-translated to (edge_i, edge_j) node ids on the host
    (index-only preprocessing); the edge RBF is recomputed per triplet on
    device from gathered node positions.
  * All gathers are indirect DMAs of 12-byte node/group position rows.
  * Output is written densely in the permuted edge order and un-permuted on
    the host during unsharding.

All floating-point math runs on device in fp32.
"""
import os
import sys
import types

sys.path.insert(0, "/opt/trn_rl_repo")

import numpy as np

# ---------------------------------------------------------------------------
# Environment patches (walrus single-sync-wait limit + NTFF profiling hook)
# ---------------------------------------------------------------------------


def _install_ntff_shim():
    if "antenv.axon_hooks" in sys.modules:
        return
    mod = types.ModuleType("antenv.axon_hooks")
    mod._hook = None
    mod.set_axon_ntff_profile_hook = lambda h: setattr(mod, "_hook", h)
    mod.get_axon_ntff_profile_hook = lambda: mod._hook
    sys.modules["antenv.axon_hooks"] = mod
    try:
        from trn_agent_boot.trn_boot import _ntff_profile_via_ctypes

        mod._hook = _ntff_profile_via_ctypes("/opt/axon/libaxon_pjrt.so")
    except Exception:
        pass


def _patch_tile():
    import bass_rust
    import concourse.tile as tile_mod
    from concourse.vector_clock import ScopedClock

    def _drain_and_barrier(self, tick_clock, wait_clock):
        nc = self.nc
        drain_inst = nc.sync.drain()
        wait_clock.add_sem_waits(
            drain_inst.ins, ScopedClock({None: tick_clock.global_clock})
        )
        si = drain_inst.ins.sync_info
        if si is not None and si.on_wait and len(si.on_wait) > 1:
            waits = list(si.on_wait)
            si.on_wait = waits[:1]
            for i, w in enumerate(waits[1:]):
                nop = nc.sync.nop(hint=f"tail_wait_{i}", nofuse=True)
                nop.ins.sync_info = bass_rust.SyncInfo(on_wait=[w], on_update=[])
        nc.all_engine_barrier()
        assert self.sems is not None
        popped = nc._tile_sem_poison_stack.pop()
        assert popped is self._sem_poison
        nc.clear_and_free_semaphores(list(self.sems.allocated().values()))
        nc.all_engine_barrier()

    tile_mod.TileContext._drain_and_barrier = _drain_and_barrier


def _split_multiwait(nc):
    """This walrus build rejects instructions with >1 sync wait; hoist
    extras onto single-wait NoOps preceding the instruction."""
    import concourse.mybir as mybir

    for fn in nc.m.functions:
        for bb in fn.blocks:
            out, changed = [], False
            for inst in bb.instructions:
                si = inst.sync_info
                w = list(si.on_wait) if (si is not None and si.on_wait) else []
                if len(w) > 1 and inst.engine != mybir.EngineType.Unassigned:
                    changed = True
                    for j, sw in enumerate(w[:-1]):
                        out.append(
                            mybir.InstNoOp(
                                name=f"{inst.name}-sw{j}",
                                engine=inst.engine,
                                sync_info=mybir.SyncInfo(on_wait=[sw], on_update=[]),
                                bass_nofuse=True,
                            )
                        )
                    si.on_wait = [w[-1]]
                out.append(inst)
            if changed:
                bb.instructions = out


_install_ntff_shim()
_patch_tile()

# ---------------------------------------------------------------------------
# Problem constants (hardcoded per the harness contract)
# ---------------------------------------------------------------------------
CUTOFF = 5.0
NR = 6           # NUM_RADIAL
NS = 7           # NUM_SPHERICAL
H = 128          # HIDDEN
EPS = 1e-6
N_NODES, N_GROUPS, N_EDGES, N_TRIP = 50000, 10000, 500000, 1000000
NCORES = 8
E_LOC = N_EDGES // NCORES            # 62500 edges per core
P = 128
ECOLS = (E_LOC + P - 1) // P         # 489 edge columns (62592 padded edges)
C = 240                              # triplet-stream columns per chunk
NF = NR + NR * NS + 1                # 49 features: rbf(6) + sbf(42) + ones

TRACE = bool(int(os.environ.get("KERNEL_TRACE", "0")))
last_exec_time_ns = None
last_scope_times = None

# Legendre/Chebyshev coefficients with sqrt((2l+1)/4pi) folded in.
_CO = np.sqrt((2.0 * np.arange(NS) + 1.0) / (4.0 * np.pi)).astype(np.float64)


# ---------------------------------------------------------------------------
# Host-side sharding / layout
# ---------------------------------------------------------------------------
def _prep_host(inputs):
    tri_e = inputs["id_reduce_ji"].astype(np.int64)
    owner = tri_e // E_LOC
    per_core = []
    for c in range(NCORES):
        sel = np.nonzero(owner == c)[0]
        le = (tri_e[sel] - c * E_LOC).astype(np.int64)
        counts = np.bincount(le, minlength=E_LOC)
        order = np.argsort(-counts, kind="stable")      # new pos -> old local id
        inv = np.empty(E_LOC, dtype=np.int64)
        inv[order] = np.arange(E_LOC)
        newid = inv[le]                                  # per-triplet new edge id
        # rank of each triplet within its edge
        sort_by_edge = np.argsort(newid, kind="stable")
        sorted_new = newid[sort_by_edge]
        first = np.searchsorted(sorted_new, sorted_new, side="left")
        rank_sorted = np.arange(len(sel)) - first
        rank = np.empty(len(sel), dtype=np.int64)
        rank[sort_by_edge] = rank_sorted
        per_core.append(dict(sel=sel, newid=newid, rank=rank,
                             counts=counts, order=order))

    n_slots = max(int(pc["rank"].max()) + 1 if len(pc["rank"]) else 1
                  for pc in per_core)
    # slot capacity c_s = #edges with count > s, uniform max across cores,
    # padded to a multiple of 128
    slot_cap = []
    for s in range(n_slots):
        cs = max(int(np.sum(pc["counts"] > s)) for pc in per_core)
        slot_cap.append(((cs + P - 1) // P) * P)
    slot_base = np.concatenate([[0], np.cumsum(slot_cap)]).astype(np.int64)
    t_pad = int(slot_base[-1])
    tcols = (t_pad + P - 1) // P
    # pad stream to full 128 columns
    t_pad = tcols * P

    return per_core, n_slots, slot_cap, slot_base, t_pad, tcols


def _find_far_pair(node_pos):
    # a node pair with distance > CUTOFF, used for zero-contribution padding
    for j in range(1, 200):
        if np.linalg.norm(node_pos[0] - node_pos[j]) > CUTOFF + 0.5:
            return 0, j
    d = np.linalg.norm(node_pos[:500] - node_pos[0], axis=1)
    return 0, int(np.argmax(d))


def _core_arrays(inputs, pc, n_slots, slot_cap, slot_base, t_pad, far_pair):
    """Build the [128, tcols] int32 gather-index streams for one core."""
    sel, newid, rank = pc["sel"], pc["newid"], pc["rank"]
    pos = slot_base[rank] + newid                      # stream position
    fi, fj = far_pair

    g = {}
    for name, full in (
        ("g_tj", inputs["triplet_j"][sel]),
        ("g_tk", inputs["triplet_k"][sel]),
        ("g_ti", inputs["triplet_i"][sel]),
        ("g_ei", inputs["edge_i"][inputs["id_expand_kj"][sel]]),
        ("g_ej", inputs["edge_j"][inputs["id_expand_kj"][sel]]),
    ):
        arr = np.zeros(t_pad, dtype=np.int32)
        if name == "g_ei":
            arr[:] = fi
        elif name == "g_ej":
            arr[:] = fj
        arr[pos] = full.astype(np.int32)
        g[name] = np.ascontiguousarray(arr.reshape(-1, P).T)   # [128, tcols]

    # owned edges, new order, padded to ECOLS*128
    order = pc["order"]
    e_pad = np.full(ECOLS * P, fi, dtype=np.int32)
    e_pad2 = np.full(ECOLS * P, fj, dtype=np.int32)
    core_idx = pc["core_idx"]
    e_i_loc = inputs["edge_i"][core_idx * E_LOC + order].astype(np.int32)
    e_j_loc = inputs["edge_j"][core_idx * E_LOC + order].astype(np.int32)
    e_pad[:E_LOC] = e_i_loc
    e_pad2[:E_LOC] = e_j_loc
    g["e_i"] = np.ascontiguousarray(e_pad.reshape(-1, P).T)    # [128, ECOLS]
    g["e_j"] = np.ascontiguousarray(e_pad2.reshape(-1, P).T)
    return g


# ---------------------------------------------------------------------------
# Device program
# ---------------------------------------------------------------------------
def _build_program(tcols, n_slots, slot_cap, slot_base, split=True):
    import concourse.bass as bass
    import concourse.mybir as mybir
    import concourse.tile as tile
    from concourse.masks import make_identity

    DT = mybir.dt.float32
    IDT = mybir.dt.int32
    AO = mybir.AluOpType
    AF = mybir.ActivationFunctionType

    nc = bass.Bass()
    # register the pi/2 const AP needed as Sin bias (cos via phase shift)
    _halfpi = float(np.pi / 2)
    _cst = nc.alloc_sbuf_tensor("const-f32-halfpi", [128, 1], DT)
    nc.gpsimd.memset(_cst.ap(), _halfpi)
    nc.const_aps.aps[(DT, _halfpi)] = _cst.ap()
    node_pos = nc.declare_dram_parameter("node_pos", [N_NODES, 3], DT, isOutput=False)
    group_pos = nc.declare_dram_parameter("group_pos", [N_GROUPS, 3], DT, isOutput=False)
    wcat = nc.declare_dram_parameter("wcat", [NF, H], DT, isOutput=False)
    dr = {}
    for nm in ("g_tj", "g_tk", "g_ti", "g_ei", "g_ej"):
        dr[nm] = nc.declare_dram_parameter(nm, [P, tcols], IDT, isOutput=False)
    for nm in ("e_i", "e_j"):
        dr[nm] = nc.declare_dram_parameter(nm, [P, ECOLS], IDT, isOutput=False)
    out = nc.declare_dram_parameter("out", [P, ECOLS, H], DT, isOutput=True)

    # slot col ranges in the stream
    slot_cols = [(int(slot_base[s]) // P, int(slot_base[s + 1]) // P)
                 for s in range(n_slots)]

    with tile.TileContext(nc) as tc:
        with (
            tc.tile_pool(name="persist", bufs=1) as pp,
            tc.tile_pool(name="idx", bufs=1) as ip,
            tc.tile_pool(name="gat", bufs=2) as gp,
            tc.tile_pool(name="pl", bufs=1) as plp,
            tc.tile_pool(name="edge", bufs=2) as ep,
            tc.tile_pool(name="psum", bufs=2, space="PSUM") as psp,
            tc.tile_pool(name="psum_t", bufs=2, space="PSUM") as pst,
            tc.tile_pool(name="stage", bufs=2) as sp,
        ):
            acc = pp.tile([P, NF, ECOLS], DT)
            nc.vector.memset(acc[:], 0.0)
            nc.vector.memset(acc[:, NF - 1, :], 1.0)     # ones feature (bias)

            wc = pp.tile([NF, H], DT)
            nc.sync.dma_start(out=wc[:], in_=wcat[:])
            ident = pp.tile([P, P], DT)
            make_identity(nc, ident[:])

            ei_t = pp.tile([P, ECOLS], IDT)
            nc.sync.dma_start(out=ei_t[:], in_=dr["e_i"][:])
            ej_t = pp.tile([P, ECOLS], IDT)
            nc.sync.dma_start(out=ej_t[:], in_=dr["e_j"][:])

            def gather(dst, table, idx_ap):
                nc.gpsimd.indirect_dma_start(
                    out=dst, out_offset=None, in_=table[:],
                    in_offset=bass.IndirectOffsetOnAxis(ap=idx_ap, axis=0),
                )

            # ---------------- edge-phase RBF into acc[:, 0:6, :] -----------
            # (independent of the triplet phase; scheduler may overlap)
            EC = 64
            with nc.named_scope("edge_rbf"):
                for g0 in range(0, ECOLS, EC):
                    g1 = min(g0 + EC, ECOLS)
                    w = g1 - g0
                    pi = ep.tile([P, EC, 3], DT, tag="pi")
                    pj = ep.tile([P, EC, 3], DT, tag="pj")
                    for _j in range(w):
                        gather(pi[:, _j], node_pos, ei_t[:, g0 + _j:g0 + _j + 1])
                        gather(pj[:, _j], node_pos, ej_t[:, g0 + _j:g0 + _j + 1])
                    _rbf(nc, plp, pi, pj, w, acc[:, 0:NR, g0:g1], AO, AF, DT,
                         tag="e")

            # ---------------- triplet phase --------------------------------
            with nc.named_scope("triplets"):
                for k0 in range(0, tcols, C):
                    k1 = min(k0 + C, tcols)
                    w = k1 - k0
                    idx = {}
                    for nm in ("g_tj", "g_tk", "g_ti", "g_ei", "g_ej"):
                        it = ip.tile([P, C], IDT, tag=nm)
                        nc.sync.dma_start(out=it[:, :w], in_=dr[nm][:, k0:k1])
                        idx[nm] = it
                    gj = gp.tile([P, C, 3], DT, tag="gj")
                    gk = gp.tile([P, C, 3], DT, tag="gk")
                    gi = gp.tile([P, C, 3], DT, tag="gi")
                    pi = gp.tile([P, C, 3], DT, tag="tpi")
                    pj = gp.tile([P, C, 3], DT, tag="tpj")
                    for _j in range(w):
                        gather(gj[:, _j], node_pos, idx["g_tj"][:, _j:_j + 1])
                        gather(gk[:, _j], node_pos, idx["g_tk"][:, _j:_j + 1])
                        gather(gi[:, _j], group_pos, idx["g_ti"][:, _j:_j + 1])
                        gather(pi[:, _j], node_pos, idx["g_ei"][:, _j:_j + 1])
                        gather(pj[:, _j], node_pos, idx["g_ej"][:, _j:_j + 1])

                    # --- geometry -> cbf planes (7) ---
                    cbf = _angles_cbf(nc, plp, gj, gk, gi, w, AO, AF, DT)
                    # --- rbf planes (6) ---
                    rbf = plp.tile([P, NR, C], DT, tag="t_rbf")
                    _rbf(nc, plp, pi, pj, w, rbf[:, :, :w], AO, AF, DT, tag="t")

                    # --- outer product + slot-prefix adds ---
                    segs = []
                    for s, (a, b) in enumerate(slot_cols):
                        lo, hi = max(a, k0), min(b, k1)
                        if lo < hi:
                            segs.append((lo - k0, hi - k0, lo - a))
                    prod = plp.tile([P, C], DT, tag="prod")
                    for l in range(NS):
                        for n in range(NR):
                            f = NR + l * NR + n
                            nc.vector.tensor_tensor(
                                out=prod[:, :w], in0=cbf[:, l, :w],
                                in1=rbf[:, n, :w], op=AO.mult)
                            for (s0, s1, e0) in segs:
                                nc.vector.tensor_tensor(
                                    out=acc[:, f, e0:e0 + (s1 - s0)],
                                    in0=acc[:, f, e0:e0 + (s1 - s0)],
                                    in1=prod[:, s0:s1], op=AO.add)

            # ---------------- edge phase: transpose + matmul + write -------
            SC = 8
            with nc.named_scope("edge_mm"):
                for j0 in range(0, ECOLS, SC):
                    j1 = min(j0 + SC, ECOLS)
                    stg = sp.tile([P, SC, H], DT, tag="stg")
                    for j in range(j0, j1):
                        ps_t = pst.tile([P, P], DT, tag="pt")
                        nc.tensor.transpose(
                            out=ps_t[:NF, :], in_=acc[:, :, j], identity=ident[:])
                        ft = plp.tile([NF, P], DT, tag="featT")
                        nc.scalar.copy(out=ft[:], in_=ps_t[:NF, :])
                        ps_o = psp.tile([P, H], DT, tag="po")
                        nc.tensor.matmul(
                            out=ps_o[:], lhsT=ft[:], rhs=wc[:],
                            start=True, stop=True)
                        nc.vector.tensor_copy(
                            out=stg[:, j - j0, :], in_=ps_o[:])
                    nc.sync.dma_start(
                        out=out[:, j0:j1, :], in_=stg[:, : j1 - j0, :])

    if split:
        _split_multiwait(nc)
    return nc


def _rbf(nc, plp, pi, pj, w, dst, AO, AF, DT, tag):
    """rbf planes from endpoint position tiles pi/pj ([P, C, 3]).
    dst: AP [P, NR, w] written with the 6 radial features."""
    import concourse.mybir as mybir

    Cw = pi.shape[1]
    t = lambda nm: plp.tile([P, Cw], DT, tag=f"{tag}_{nm}", name=f"{tag}_{nm}")
    dx, dy, dz = t("dx"), t("dy"), t("dz")
    for c, d in enumerate((dx, dy, dz)):
        nc.vector.tensor_tensor(out=d[:, :w], in0=pi[:, :w, c],
                                in1=pj[:, :w, c], op=AO.subtract)
    dd = t("dd")
    tmp2 = t("tmp2")
    nc.vector.tensor_tensor(out=dd[:, :w], in0=dx[:, :w], in1=dx[:, :w],
                            op=AO.mult)
    for d in (dy, dz):
        nc.vector.tensor_tensor(out=tmp2[:, :w], in0=d[:, :w], in1=d[:, :w],
                                op=AO.mult)
        nc.vector.tensor_tensor(out=dd[:, :w], in0=dd[:, :w], in1=tmp2[:, :w],
                                op=AO.add)
    d_ = t("d")
    nc.scalar.activation(out=d_[:, :w], in_=dd[:, :w], func=AF.Sqrt)
    dm = t("dm")
    nc.vector.tensor_scalar_max(out=dm[:, :w], in0=d_[:, :w], scalar1=EPS)
    # ds clamped to 1: the envelope masks ds>1 to zero, so the sines only
    # matter on [0,1] -- keeps ACT Sin args within its [-pi, pi] range.
    dsc = t("dsc")
    nc.vector.tensor_scalar(out=dsc[:, :w], in0=dm[:, :w],
                            scalar1=float(1.0 / CUTOFF), scalar2=1.0,
                            op0=AO.mult, op1=AO.min)
    s1 = t("s1")
    nc.scalar.activation(out=s1[:, :w], in_=dsc[:, :w], func=AF.Sin,
                         scale=float(np.pi))
    sh = t("sh")
    nc.scalar.activation(out=sh[:, :w], in_=dsc[:, :w], func=AF.Sin,
                         scale=float(np.pi / 2))
    c1 = t("c1")
    nc.vector.tensor_tensor(out=c1[:, :w], in0=sh[:, :w], in1=sh[:, :w],
                            op=AO.mult)
    nc.vector.tensor_scalar(out=c1[:, :w], in0=c1[:, :w], scalar1=-2.0,
                            scalar2=1.0, op0=AO.mult, op1=AO.add)
    me = t("me")
    nc.vector.tensor_scalar(out=me[:, :w], in0=dm[:, :w], scalar1=float(CUTOFF),
                            scalar2=None, op0=AO.is_le)
    env = t("env")
    nc.vector.tensor_scalar(out=env[:, :w], in0=c1[:, :w], scalar1=0.5,
                            scalar2=0.5, op0=AO.mult, op1=AO.add)
    nc.vector.tensor_tensor(out=env[:, :w], in0=env[:, :w], in1=me[:, :w],
                            op=AO.mult)
    ds = t("ds")
    nc.vector.tensor_scalar(out=ds[:, :w], in0=dm[:, :w],
                            scalar1=float(1.0 / CUTOFF), scalar2=EPS,
                            op0=AO.mult, op1=AO.max)
    gi_ = t("gi_")
    nc.vector.reciprocal(out=gi_[:, :w], in_=ds[:, :w])
    eg = t("eg")
    nc.vector.tensor_tensor(out=eg[:, :w], in0=env[:, :w], in1=gi_[:, :w],
                            op=AO.mult)
    # sin recurrence
    twoc = t("twoc")
    nc.vector.tensor_scalar_mul(out=twoc[:, :w], in0=c1[:, :w], scalar1=2.0)
    sins = [s1]
    sprev, scur = None, s1
    for k in range(2, NR + 1):
        sn = t(f"sin{k}")
        nc.vector.tensor_tensor(out=sn[:, :w], in0=twoc[:, :w],
                                in1=scur[:, :w], op=AO.mult)
        if sprev is not None:
            nc.vector.tensor_tensor(out=sn[:, :w], in0=sn[:, :w],
                                    in1=sprev[:, :w], op=AO.subtract)
        sins.append(sn)
        sprev, scur = scur, sn
    for n in range(NR):
        nc.vector.tensor_tensor(out=dst[:, n, :], in0=eg[:, :w],
                                in1=sins[n][:, :w], op=AO.mult)


def _angles_cbf(nc, plp, gj, gk, gi, w, AO, AF, DT):
    """Geometry -> cbf tile [P, NS, C] (coeffs folded)."""
    Cw = gj.shape[1]
    t = lambda nm: plp.tile([P, Cw], DT, tag=f"a_{nm}", name=f"a_{nm}")
    r1 = [t(f"r1{c}") for c in "xyz"]
    r2 = [t(f"r2{c}") for c in "xyz"]
    for c in range(3):
        nc.vector.tensor_tensor(out=r1[c][:, :w], in0=gj[:, :w, c],
                                in1=gi[:, :w, c], op=AO.subtract)
        nc.vector.tensor_tensor(out=r2[c][:, :w], in0=gk[:, :w, c],
                                in1=gi[:, :w, c], op=AO.subtract)
    s1, s2, d12 = t("s1"), t("s2"), t("d12")
    tmp = t("tmp")
    for dst, a, b in ((s1, r1, r1), (s2, r2, r2), (d12, r1, r2)):
        nc.vector.tensor_tensor(out=dst[:, :w], in0=a[0][:, :w],
                                in1=b[0][:, :w], op=AO.mult)
        for c in (1, 2):
            nc.vector.tensor_tensor(out=tmp[:, :w], in0=a[c][:, :w],
                                    in1=b[c][:, :w], op=AO.mult)
            nc.vector.tensor_tensor(out=dst[:, :w], in0=dst[:, :w],
                                    in1=tmp[:, :w], op=AO.add)
    # cross product squared norm
    cr2 = t("cr2")
    first = True
    for (a, b), (cc, dd_) in (((1, 2), (2, 1)), ((2, 0), (0, 2)), ((0, 1), (1, 0))):
        u, v = t("cu"), t("cv")
        nc.vector.tensor_tensor(out=u[:, :w], in0=r1[a][:, :w],
                                in1=r2[b][:, :w], op=AO.mult)
        nc.vector.tensor_tensor(out=v[:, :w], in0=r1[cc][:, :w],
                                in1=r2[dd_][:, :w], op=AO.mult)
        nc.vector.tensor_tensor(out=u[:, :w], in0=u[:, :w], in1=v[:, :w],
                                op=AO.subtract)
        nc.vector.tensor_tensor(out=u[:, :w], in0=u[:, :w], in1=u[:, :w],
                                op=AO.mult)
        if first:
            nc.vector.tensor_copy(out=cr2[:, :w], in_=u[:, :w])
            first = False
        else:
            nc.vector.tensor_tensor(out=cr2[:, :w], in0=cr2[:, :w],
                                    in1=u[:, :w], op=AO.add)
    a1, a2 = t("a1"), t("a2")
    nc.scalar.activation(out=a1[:, :w], in_=s1[:, :w], func=AF.Sqrt)
    nc.scalar.activation(out=a2[:, :w], in_=s2[:, :w], func=AF.Sqrt)
    p = t("p")
    nc.vector.tensor_scalar_add(out=p[:, :w], in0=a1[:, :w], scalar1=EPS)
    nc.vector.tensor_scalar_add(out=tmp[:, :w], in0=a2[:, :w], scalar1=EPS)
    nc.vector.tensor_tensor(out=p[:, :w], in0=p[:, :w], in1=tmp[:, :w],
                            op=AO.mult)
    q = t("q")
    nc.vector.reciprocal(out=q[:, :w], in_=p[:, :w])
    x = t("x")
    nc.vector.tensor_tensor(out=x[:, :w], in0=d12[:, :w], in1=q[:, :w],
                            op=AO.mult)
    nc.vector.tensor_scalar(out=x[:, :w], in0=x[:, :w],
                            scalar1=float(1.0 - EPS), scalar2=float(-1.0 + EPS),
                            op0=AO.min, op1=AO.max)
    y2 = t("y2")
    nc.vector.tensor_tensor(out=y2[:, :w], in0=q[:, :w], in1=q[:, :w],
                            op=AO.mult)
    nc.vector.tensor_tensor(out=y2[:, :w], in0=y2[:, :w], in1=cr2[:, :w],
                            op=AO.mult)
    z = t("z")
    nc.vector.tensor_tensor(out=z[:, :w], in0=x[:, :w], in1=x[:, :w],
                            op=AO.mult)
    nc.vector.tensor_tensor(out=z[:, :w], in0=z[:, :w], in1=y2[:, :w],
                            op=AO.add)
    wr = t("wr")
    nc.scalar.activation(out=wr[:, :w], in_=z[:, :w], func=AF.Sqrt)
    wi = t("wi")
    nc.vector.reciprocal(out=wi[:, :w], in_=wr[:, :w])
    cz = t("cz")
    nc.vector.tensor_tensor(out=cz[:, :w], in0=x[:, :w], in1=wi[:, :w],
                            op=AO.mult)
    # too_close mask: (a1 + EPS < 1e-4) | (a2 + EPS < 1e-4)  -> c = 0
    m1, m2 = t("m1"), t("m2")
    thr = float(1e-4 - EPS)
    nc.vector.tensor_scalar(out=m1[:, :w], in0=a1[:, :w], scalar1=thr,
                            scalar2=None, op0=AO.is_lt)
    nc.vector.tensor_scalar(out=m2[:, :w], in0=a2[:, :w], scalar1=thr,
                            scalar2=None, op0=AO.is_lt)
    nc.vector.tensor_tensor(out=m1[:, :w], in0=m1[:, :w], in1=m2[:, :w],
                            op=AO.max)
    # cz *= (1 - mask)   (mask is 0/1 fp32)
    nc.vector.tensor_scalar(out=m1[:, :w], in0=m1[:, :w], scalar1=-1.0,
                            scalar2=1.0, op0=AO.mult, op1=AO.add)
    nc.vector.tensor_tensor(out=cz[:, :w], in0=cz[:, :w], in1=m1[:, :w],
                            op=AO.mult)

    # Legendre / Chebyshev planes with coeffs folded
    cbf = plp.tile([P, NS, Cw], DT, tag="cbf")
    c2 = t("c2")
    nc.vector.tensor_tensor(out=c2[:, :w], in0=cz[:, :w], in1=cz[:, :w],
                            op=AO.mult)
    co = _CO
    nc.vector.memset(cbf[:, 0, :], float(co[0]))
    nc.vector.tensor_scalar_mul(out=cbf[:, 1, :w], in0=cz[:, :w],
                                scalar1=float(co[1]))
    nc.vector.tensor_scalar(out=cbf[:, 2, :w], in0=c2[:, :w],
                            scalar1=float(1.5 * co[2]),
                            scalar2=float(-0.5 * co[2]),
                            op0=AO.mult, op1=AO.add)
    nc.vector.tensor_scalar(out=tmp[:, :w], in0=c2[:, :w],
                            scalar1=float(2.5 * co[3]),
                            scalar2=float(-1.5 * co[3]),
                            op0=AO.mult, op1=AO.add)
    nc.vector.tensor_tensor(out=cbf[:, 3, :w], in0=tmp[:, :w], in1=cz[:, :w],
                            op=AO.mult)
    # P4 = co4*(4.375 c^4 - 3.75 c^2 + 0.375)
    nc.vector.tensor_scalar(out=tmp[:, :w], in0=c2[:, :w],
                            scalar1=float(4.375 * co[4]),
                            scalar2=float(-3.75 * co[4]),
                            op0=AO.mult, op1=AO.add)
    nc.vector.tensor_tensor(out=tmp[:, :w], in0=tmp[:, :w], in1=c2[:, :w],
                            op=AO.mult)
    nc.vector.tensor_scalar_add(out=cbf[:, 4, :w], in0=tmp[:, :w],
                                scalar1=float(0.375 * co[4]))
    # P5 = co5*c*(7.875 c^4 - 8.75 c^2 + 1.875)
    nc.vector.tensor_scalar(out=tmp[:, :w], in0=c2[:, :w],
                            scalar1=float(7.875 * co[5]),
                            scalar2=float(-8.75 * co[5]),
                            op0=AO.mult, op1=AO.add)
    nc.vector.tensor_tensor(out=tmp[:, :w], in0=tmp[:, :w], in1=c2[:, :w],
                            op=AO.mult)
    nc.vector.tensor_scalar_add(out=tmp[:, :w], in0=tmp[:, :w],
                                scalar1=float(1.875 * co[5]))
    nc.vector.tensor_tensor(out=cbf[:, 5, :w], in0=tmp[:, :w], in1=cz[:, :w],
                            op=AO.mult)
    # P6 = co6*(32 c^6 - 48 c^4 + 18 c^2 - 1)   (cos(6θ) = T6(cosθ))
    nc.vector.tensor_scalar(out=tmp[:, :w], in0=c2[:, :w],
                            scalar1=float(32.0 * co[6]),
                            scalar2=float(-48.0 * co[6]),
                            op0=AO.mult, op1=AO.add)
    nc.vector.tensor_tensor(out=tmp[:, :w], in0=tmp[:, :w], in1=c2[:, :w],
                            op=AO.mult)
    nc.vector.tensor_scalar_add(out=tmp[:, :w], in0=tmp[:, :w],
                                scalar1=float(18.0 * co[6]))
    nc.vector.tensor_tensor(out=tmp[:, :w], in0=tmp[:, :w], in1=c2[:, :w],
                            op=AO.mult)
    nc.vector.tensor_scalar_add(out=cbf[:, 6, :w], in0=tmp[:, :w],
                                scalar1=float(-1.0 * co[6]))
    return cbf


# ---------------------------------------------------------------------------
# Entry point
# ---------------------------------------------------------------------------
def kernel(**inputs):
    global last_exec_time_ns, last_scope_times
    from concourse.bass_utils import run_bass_kernel_spmd

    inputs = {k: np.asarray(v) for k, v in inputs.items()}
    per_core, n_slots, slot_cap, slot_base, t_pad, tcols = _prep_host(inputs)
    far_pair = _find_far_pair(inputs["node_pos"])

    wcat = np.concatenate(
        [inputs["W_rbf"], inputs["W_sbf"], inputs["b_rbf"][None, :]], axis=0
    ).astype(np.float32)

    in_maps = []
    for c in range(NCORES):
        pc = per_core[c]
        pc["core_idx"] = c
        g = _core_arrays(inputs, pc, n_slots, slot_cap, slot_base, t_pad,
                         far_pair)
        m = {
            "node_pos": inputs["node_pos"].astype(np.float32),
            "group_pos": inputs["group_pos"].astype(np.float32),
            "wcat": wcat,
        }
        m.update(g)
        in_maps.append(m)

    nc = _build_program(tcols, n_slots, slot_cap, slot_base)
    res = run_bass_kernel_spmd(nc, in_maps, list(range(NCORES)), trace=TRACE)
    last_exec_time_ns = res.exec_time_ns
    last_scope_times = res.per_core_scope_times

    out_full = np.empty((N_EDGES, H), dtype=np.float32)
    for c in range(NCORES):
        o = res.results[c]["out"]                      # [128, ECOLS, H]
        rows = np.ascontiguousarray(o.transpose(1, 0, 2)).reshape(-1, H)
        order = per_core[c]["order"]
        blk = np.empty((E_LOC, H), dtype=np.float32)
        blk[order] = rows[:E_LOC]
        out_full[c * E_LOC:(c + 1) * E_LOC] = blk
    return out_full
